# revision 38
# baseline (speedup 1.0000x reference)
"""TRN2 Bass kernel for BEiT-style attention (nn_Attention_27771258536423).

Data-parallel over batch: 8 batches/core. Per core (ntok = 8*197 = 1576):
  P1: qkv projection, all bf16. q,k channel-major [ch, tok] (q pre-scaled
      + biased), v token-major per batch in a head-major [*, 12*65] layout
      with a ones column per head (for softmax denominators).
  P2 per (batch, head-pair): scores computed TRANSPOSED S^T[n2, n1] = K.Q
      -> +bias^T (host-gathered) -> exp (scalar engine, no accum)
      -> AV with E^T as stationary: out[n1, 65] (col 64 = sum = denom)
      -> reciprocal + per-partition scale -> token-major attn [tok, 768]
      -> DMA-transpose engine flips to channel-major attT for P3.
  P3: y = attT.T @ proj_w.T + proj_b, token-major f32 out.
"""
import sys

sys.path.insert(0, '/opt/trn_rl_repo')

import numpy as np
import ml_dtypes

import concourse.bass as bass
import concourse.mybir as mybir
import concourse.tile as tile
from concourse import bacc

dt = mybir.dt
BF16 = ml_dtypes.bfloat16

DIM = 768
NH = 12
HD = 64
N_TOK = 197
SCALE = HD ** (-0.5)
NB = 8                     # batches per core
NTOK = NB * N_TOK          # 1576
QKW = 1664                 # padded qk tile width (13*128)
BSTRIDE = 1664             # attT block stride: 8 batches x 208 (16-aligned slots)
N1C = [(0, 128), (128, 69)]   # token chunks within one batch

_cache = {}


def _ap(t, offset, ap):
    return bass.AP(tensor=t.tensor if hasattr(t, 'tensor') else t,
                   offset=offset, ap=ap)


def build_program():
    nc = bacc.Bacc(None)

    xTb_d = nc.dram_tensor("xTb", [DIM, NTOK], dt.bfloat16, kind="ExternalInput")
    wqkvTb_d = nc.dram_tensor("wqkvTb", [DIM, 3 * DIM], dt.bfloat16, kind="ExternalInput")
    wprojTb_d = nc.dram_tensor("wprojTb", [DIM, DIM], dt.bfloat16, kind="ExternalInput")
    qbs_d = nc.dram_tensor("qbs", [128, 6], dt.float32, kind="ExternalInput")
    vb_d = nc.dram_tensor("vb", [DIM], dt.float32, kind="ExternalInput")
    pb_d = nc.dram_tensor("pb", [DIM], dt.float32, kind="ExternalInput")
    biasTp_d = nc.dram_tensor("biasTp", [6, 128, 1024], dt.bfloat16, kind="ExternalInput")
    y_d = nc.dram_tensor("y", [NTOK, DIM], dt.float32, kind="ExternalOutput")

    Exp = mybir.ActivationFunctionType.Exp
    Ident = mybir.ActivationFunctionType.Identity

    # qkv N-chunks over tokens (394 = 2 batches)
    qkv_nc = [(394 * i, 394) for i in range(4)]

    with tile.TileContext(nc) as tc:
        import contextlib
        with contextlib.ExitStack() as stk:
            consts = stk.enter_context(tc.tile_pool(name="consts", bufs=1))
            qkp = stk.enter_context(tc.tile_pool(name="qkp", bufs=1))
            vp = stk.enter_context(tc.tile_pool(name="vp", bufs=1))
            xwp = stk.enter_context(tc.tile_pool(name="xwp", bufs=1))
            atp = stk.enter_context(tc.tile_pool(name="atp", bufs=1))

            # ---------- constants / inputs ----------
            # small consts first (sync queue), then x on sync / w on scalar
            # queue so the k-ordered accumulation chains pipeline behind the
            # loads
            qbs_sb = consts.tile([128, 6], dt.float32, name="qbs", tag="qbs")
            nc.sync.dma_start(out=qbs_sb[:, :], in_=qbs_d[:, :])
            vb_rep = consts.tile([128, DIM], dt.float32, name="vbrep", tag="vbrep")
            pb_rep = consts.tile([128, DIM], dt.float32, name="pbrep", tag="pbrep")
            x_sb = []
            w_sb = []
            for k in range(6):
                xt = xwp.tile([128, NTOK], dt.bfloat16, name=f"x{k}", tag=f"x{k}")
                x_sb.append(xt)
                wt = xwp.tile([128, 3 * DIM], dt.bfloat16, name=f"w{k}", tag=f"w{k}")
                w_sb.append(wt)
            # chunked loads on three queues so the k-ordered accumulation
            # chains can start as soon as the first chunks land; the scalar
            # queue gets only the k-col loads so the q-copies aren't stuck
            # behind DMA issues
            for k in range(6):
                nc.sync.dma_start(out=x_sb[k][:, 0:394],
                                  in_=xTb_d[128 * k:128 * (k + 1), 0:394])
                nc.gpsimd.dma_start(out=w_sb[k][:, 0:128],
                                    in_=wqkvTb_d[128 * k:128 * (k + 1), 0:128])
                nc.scalar.dma_start(out=w_sb[k][:, 768:896],
                                    in_=wqkvTb_d[128 * k:128 * (k + 1), 768:896])
            for k in range(6):
                nc.sync.dma_start(out=x_sb[k][:, 394:788],
                                  in_=xTb_d[128 * k:128 * (k + 1), 394:788])
                nc.gpsimd.dma_start(out=w_sb[k][:, 128:768],
                                    in_=wqkvTb_d[128 * k:128 * (k + 1), 128:768])
                nc.scalar.dma_start(out=w_sb[k][:, 896:1536],
                                    in_=wqkvTb_d[128 * k:128 * (k + 1), 896:1536])
            for k in range(6):
                nc.sync.dma_start(out=x_sb[k][:, 788:NTOK],
                                  in_=xTb_d[128 * k:128 * (k + 1), 788:NTOK])
                nc.gpsimd.dma_start(out=w_sb[k][:, 1536:2304],
                                    in_=wqkvTb_d[128 * k:128 * (k + 1), 1536:2304])
            # broadcast consts late (128-partition replication is slow)
            nc.gpsimd.dma_start(out=vb_rep[:, :],
                                in_=_ap(vb_d, 0, [[0, 128], [1, DIM]]))
            nc.gpsimd.dma_start(out=pb_rep[:, :],
                                in_=_ap(pb_d, 0, [[0, 128], [1, DIM]]))
            biasT_sb = []
            for p in range(6):
                bt = consts.tile([128, 1024], dt.bfloat16, name=f"bT{p}", tag=f"bT{p}")
                nc.sync.dma_start(out=bt[:, :], in_=biasTp_d[p, :, :])
                biasT_sb.append(bt)
            wp_sb = []
            for k in range(6):
                wt = xwp.tile([128, DIM], dt.bfloat16, name=f"wp{k}", tag=f"wp{k}")
                nc.sync.dma_start(out=wt[:, :], in_=wprojTb_d[128 * k:128 * (k + 1), :])
                wp_sb.append(wt)

            # ---------- persistent sbuf ----------
            qk_sb = []   # 0-5 q (ch-major, scaled+biased), 6-11 k (ch-major)
            for t in range(12):
                qk_sb.append(qkp.tile([128, QKW], dt.bfloat16, name=f"qk{t}", tag=f"qk{t}"))
            for t in range(6, 12):   # zero the pad cols read by b=7 extended stat
                nc.gpsimd.memset(qk_sb[t][:, NTOK:QKW], 0.0)
            v65 = []     # per batch: [128, 780] + [69, 780]; head h cols 65h..65h+64, ones at 65h+64
            for b in range(NB):
                v65.append([vp.tile([128, 780], dt.bfloat16, name=f"v{b}_0", tag=f"v{b}_0"),
                            vp.tile([69, 780], dt.bfloat16, name=f"v{b}_1", tag=f"v{b}_1")])
            attT = atp.tile([128, 6 * BSTRIDE], dt.bfloat16, name="attT", tag="attT")

            mps = stk.enter_context(tc.tile_pool(name="mps", bufs=3, space="PSUM"))
            qkps_box = {}

            def emit_qk_tile(m, chunks):
                qkps = qkps_box['p']
                # qk channel tile m: q if m<6 else k; stat = w cols 128m..
                for (no, nw) in chunks:
                    ps = qkps.tile([128, 394], dt.float32, name="qkps", tag="qkps")
                    for k in range(6):
                        nc.tensor.matmul(ps[:, 0:nw],
                                         w_sb[k][:, 128 * m:128 * (m + 1)],
                                         x_sb[k][:, no:no + nw],
                                         start=(k == 0), stop=(k == 5))
                    if m < 6:
                        # q: (ps*SCALE + qb*SCALE) on scalar engine
                        nc.scalar.activation(out=qk_sb[m][:, no:no + nw],
                                             in_=ps[:, 0:nw], func=Ident,
                                             bias=qbs_sb[:, m:m + 1],
                                             scale=float(SCALE))
                    else:
                        nc.vector.tensor_copy(qk_sb[m][:, no:no + nw], ps[:, 0:nw])

            def emit_v65(b, part):
                # part 0: ci=0 halves; part 1: ci=1 halves (+ ones memsets)
                ci = part
                to, tw_ = N1C[ci]
                vt = v65[b][ci]
                for half in range(2):
                    ps = mps.tile([128, 384], dt.float32, name="mp", tag="mp")
                    for k in range(6):
                        nc.tensor.matmul(
                            ps[0:tw_, 0:384],
                            x_sb[k][:, N_TOK * b + to:N_TOK * b + to + tw_],
                            w_sb[k][:, 1536 + 384 * half:1536 + 384 * (half + 1)],
                            start=(k == 0), stop=(k == 5))
                    # add v bias, scatter to head-major 65-stride cols
                    out_ap = _ap(vt, vt.offset + 65 * 6 * half,
                                 [[vt.ap[0][0], tw_], [65, 6], [1, 64]])
                    in1_ap = _ap(vb_rep, vb_rep.offset + 384 * half,
                                 [[vb_rep.ap[0][0], tw_], [64, 6], [1, 64]])
                    nc.vector.tensor_tensor(out=out_ap, in0=ps[0:tw_, 0:384],
                                            in1=in1_ap, op=mybir.AluOpType.add)
                # ones columns for this tile
                ones_ap = _ap(vt, vt.offset + 64, [[vt.ap[0][0], tw_], [65, 12]])
                nc.gpsimd.memset(ones_ap, 1.0)

            # ---------- P1: qk (+ first v65 batches) ----------
            with tc.tile_pool(name="qkps", bufs=4, space="PSUM") as qkps:
                qkps_box['p'] = qkps
                # token chunks 0-1 for all tiles first (wave-1 x data), then
                # chunks 2-3 once the second-wave DMAs have landed
                for j in range(6):
                    emit_qk_tile(j, qkv_nc[0:2])
                    emit_qk_tile(j + 6, qkv_nc[0:2])
                for j in range(6):
                    emit_qk_tile(j, qkv_nc[2:4])
                    emit_qk_tile(j + 6, qkv_nc[2:4])
                for b in range(2):
                    emit_v65(b, 0)
                    emit_v65(b, 1)

            # ---------- P2 + interleaved v65/P3 ----------
            spool = stk.enter_context(tc.tile_pool(name="spool", bufs=5, space="PSUM"))
            ssp = stk.enter_context(tc.tile_pool(name="ssp", bufs=6))
            etp = stk.enter_context(tc.tile_pool(name="etp", bufs=6))
            recp = stk.enter_context(tc.tile_pool(name="recp", bufs=6))
            attp = stk.enter_context(tc.tile_pool(name="attp", bufs=2))
            ysb = stk.enter_context(tc.tile_pool(name="ysb", bufs=3))

            def unit_head(b, h, att2):
                p, i = h // 2, h % 2
                qt = qk_sb[p]
                kt = qk_sb[6 + p]
                po = 64 * i
                sp = spool.tile([128, 512], dt.float32, name="sp", tag="sp")
                for ci in range(2):
                    nc.tensor.matmul(
                        sp[0:128, 256 * ci:256 * ci + N_TOK],
                        kt[po:po + 64,
                           N_TOK * b + 128 * ci:N_TOK * b + 128 * (ci + 1)],
                        qt[po:po + 64, N_TOK * b:N_TOK * b + N_TOK],
                        start=True, stop=True)
                # exp of raw scores (scalar engine, psum -> bf16 sbuf)
                es = ssp.tile([128, 512], dt.bfloat16, name="es", tag="es")
                sp_ap = _ap(sp, sp.offset, [[sp.ap[0][0], 128], [256, 2], [1, N_TOK]])
                es_ap = _ap(es, es.offset, [[es.ap[0][0], 128], [256, 2], [1, N_TOK]])
                nc.scalar.activation(out=es_ap, in_=sp_ap, func=Exp)
                # fold bias in via exp(s+b) = exp(s)*exp(b); all-bf16 sbuf op,
                # EBT precomputed on host
                bt = biasT_sb[p]
                et = etp.tile([128, 512], dt.bfloat16, name="et", tag="et")
                et_ap = _ap(et, et.offset, [[et.ap[0][0], 128], [256, 2], [1, N_TOK]])
                es_ap2 = _ap(es, es.offset, [[es.ap[0][0], 128], [256, 2], [1, N_TOK]])
                bt_ap = _ap(bt, bt.offset + 512 * i,
                            [[bt.ap[0][0], 128], [256, 2], [1, N_TOK]])
                eng = nc.vector
                eng.tensor_tensor(out=et_ap, in0=es_ap2, in1=bt_ap,
                                  op=mybir.AluOpType.mult)
                # AV: out[n1, 65] (col 64 = denom); psum shared with v65/P3
                ap_ = mps.tile([128, 384], dt.float32, name="mp", tag="mp")
                for cj, (n1o, n1c) in enumerate(N1C):
                    for ci, (n2o, n2c) in enumerate(N1C):
                        nc.tensor.matmul(
                            ap_[0:n1c, 65 * cj:65 * cj + 65],
                            et[0:n2c, 256 * ci + 128 * cj:256 * ci + 128 * cj + n1c],
                            v65[b][ci][0:n2c, 65 * h:65 * h + 65],
                            start=(ci == 0), stop=(ci == 1))
                rec = recp.tile([128, 2], dt.float32, name="rec", tag="rec")
                nc.vector.reciprocal(
                    rec[:, 0:2],
                    _ap(ap_, ap_.offset + 64, [[ap_.ap[0][0], 128], [65, 2]]))
                for cj, (n1o, n1c) in enumerate(N1C):
                    src = ap_[0:n1c, 65 * cj:65 * cj + 64]
                    dst = att2[cj][0:n1c, 64 * h:64 * h + 64]
                    if (b + h + cj) % 2 == 0:
                        nc.scalar.mul(dst, src, rec[0:n1c, cj:cj + 1])
                    else:
                        nc.vector.tensor_scalar(
                            out=dst, in0=src, scalar1=rec[0:n1c, cj:cj + 1],
                            scalar2=None, op0=mybir.AluOpType.mult)

            def emit_p3(b, ci):
                to, tw_ = N1C[ci]
                ys = ysb.tile([128, DIM], dt.float32, name="ys", tag="ys")
                for half in range(2):
                    pp = mps.tile([128, 384], dt.float32, name="mp", tag="mp")
                    for c in range(6):
                        nc.tensor.matmul(
                            pp[0:tw_, 0:384],
                            attT[0:128,
                                 BSTRIDE * c + 208 * b + to:
                                 BSTRIDE * c + 208 * b + to + tw_],
                            wp_sb[c][0:128, 384 * half:384 * (half + 1)],
                            start=(c == 0), stop=(c == 5))
                    nc.vector.tensor_tensor(
                        out=ys[0:tw_, 384 * half:384 * (half + 1)],
                        in0=pp[0:tw_, 0:384],
                        in1=pb_rep[0:tw_, 384 * half:384 * (half + 1)],
                        op=mybir.AluOpType.add)
                nc.sync.dma_start(out=y_d[N_TOK * b + to:N_TOK * b + to + tw_, :],
                                  in_=ys[0:tw_, :])

            for b in range(NB):
                att2 = [attp.tile([128, DIM], dt.bfloat16, name="att0", tag="att0"),
                        attp.tile([80, DIM], dt.bfloat16, name="att1", tag="att1")]
                # pad rows 69:80 of att1 (transpose reads them); partition
                # slices must start at 0/32/64/96, so clear the whole tile
                # before the scale-copies fill rows 0:69
                nc.gpsimd.memset(att2[1][0:80, :], 0.0)
                for h in range(12):
                    unit_head(b, h, att2)
                    # P3 of the previous batch goes here so the in-order PE
                    # queue doesn't stall on the previous transpose DMA
                    if h == 2 and b > 0:
                        emit_p3(b - 1, 0)
                    elif h == 5 and b > 0:
                        emit_p3(b - 1, 1)
                    if b + 2 < NB:
                        if h == 3:
                            emit_v65(b + 2, 0)
                        elif h == 7:
                            emit_v65(b + 2, 1)
                    if b == NB - 1 and h % 2 == 1:
                        # last batch: transpose block c as soon as its two
                        # heads are done, shrinking the end-of-kernel drain
                        c = h // 2
                        oc0 = _ap(attT, attT.offset + BSTRIDE * c + 208 * b,
                                  [[attT.ap[0][0], 128], [1, 128]])
                        nc.sync.dma_start_transpose(
                            out=oc0, in_=att2[0][0:128, 128 * c:128 * (c + 1)])
                        oc1 = _ap(attT, attT.offset + BSTRIDE * c + 208 * b + 128,
                                  [[attT.ap[0][0], 128], [1, 80]])
                        nc.sync.dma_start_transpose(
                            out=oc1, in_=att2[1][0:80, 128 * c:128 * (c + 1)])
                if b < NB - 1:
                    # transpose token-major att -> channel-major attT blocks
                    # (dst token offsets must be 16-aligned: 208*b, 208*b+128)
                    out0 = _ap(attT, attT.offset + 208 * b,
                               [[attT.ap[0][0], 128], [BSTRIDE, 6], [1, 128]])
                    nc.sync.dma_start_transpose(out=out0, in_=att2[0][0:128, :])
                    out1 = _ap(attT, attT.offset + 208 * b + 128,
                               [[attT.ap[0][0], 128], [BSTRIDE, 6], [1, 80]])
                    nc.sync.dma_start_transpose(out=out1, in_=att2[1][0:80, :])
            emit_p3(NB - 1, 0)
            emit_p3(NB - 1, 1)

    nc.compile()
    return nc


def _marshal(x, qkv_w, q_bias, v_bias, rpb_table, proj_w, proj_b, rel_index):
    B = x.shape[0]
    ncore = 8
    bpc = B // ncore
    x2 = np.ascontiguousarray(x.reshape(B, N_TOK, DIM))

    wqkvTb = np.ascontiguousarray(qkv_w.T.astype(BF16))
    wprojTb = np.ascontiguousarray(proj_w.T.astype(BF16))
    qbs = np.ascontiguousarray((q_bias * SCALE).astype(np.float32).reshape(6, 128).T)

    # host-side gather of exp(relative-position bias), packed transposed:
    # per head pair p: [128, 1024] with chunks at col 0/256 (h=2p) and 512/768.
    # Shipping exp(b) lets the device fold the bias as exp(s)*exp(b) with a
    # cheap all-bf16 multiply instead of a psum f32 add.
    Bfull = np.exp(rpb_table.astype(np.float64))[np.asarray(rel_index)]  # [197,197,12]
    biasTp = np.zeros((6, 128, 1024), dtype=BF16)
    for p in range(6):
        for i in range(2):
            bT = Bfull[:, :, 2 * p + i].T               # [n2, n1]
            biasTp[p, 0:128, 512 * i:512 * i + N_TOK] = bT[0:128, :].astype(BF16)
            biasTp[p, 0:69, 512 * i + 256:512 * i + 256 + N_TOK] = bT[128:197, :].astype(BF16)

    shared = {"wqkvTb": wqkvTb, "wprojTb": wprojTb, "qbs": qbs,
              "vb": np.ascontiguousarray(v_bias.astype(np.float32)),
              "pb": np.ascontiguousarray(proj_b.astype(np.float32)),
              "biasTp": biasTp}
    in_maps = []
    for c in range(ncore):
        xTb = np.ascontiguousarray(
            x2[c * bpc:(c + 1) * bpc].reshape(bpc * N_TOK, DIM).T.astype(BF16))
        m = dict(shared)
        m["xTb"] = xTb
        in_maps.append(m)
    return in_maps, bpc


last_exec_time_ns = None
last_results = None


def _install_ntff_hook():
    """Provide antenv.axon_hooks + register the ctypes NTFF hook (the agent
    image's antenv lacks axon_hooks, so trn_boot degraded silently)."""
    import types
    import contextlib
    import ctypes

    try:
        from antenv.axon_hooks import get_axon_ntff_profile_hook
        if get_axon_ntff_profile_hook() is not None:
            return
    except ImportError:
        import antenv
        mod = types.ModuleType("antenv.axon_hooks")
        mod._hook = None

        def set_axon_ntff_profile_hook(h):
            mod._hook = h

        def get_axon_ntff_profile_hook():
            return mod._hook

        mod.set_axon_ntff_profile_hook = set_axon_ntff_profile_hook
        mod.get_axon_ntff_profile_hook = get_axon_ntff_profile_hook
        sys.modules["antenv.axon_hooks"] = mod
        antenv.axon_hooks = mod

    so_path = "/opt/axon/libaxon_pjrt.so"
    lib = ctypes.CDLL(so_path)
    if not hasattr(lib, "axon_start_nrt_profile"):
        return
    lib.axon_start_nrt_profile.argtypes = [ctypes.POINTER(ctypes.c_int64),
                                           ctypes.c_size_t]
    lib.axon_start_nrt_profile.restype = ctypes.c_int64
    lib.axon_stop_nrt_profile.argtypes = [ctypes.c_char_p]
    lib.axon_stop_nrt_profile.restype = ctypes.c_int64

    @contextlib.contextmanager
    def _hook(output_dir, device_ids):
        import jax
        jax.devices()
        if device_ids:
            ids = (ctypes.c_int64 * len(device_ids))(*device_ids)
            rc = lib.axon_start_nrt_profile(ids, len(device_ids))
        else:
            rc = lib.axon_start_nrt_profile(None, 0)
        if rc != 0:
            raise RuntimeError(f"axon_start_nrt_profile rc={rc}")
        try:
            yield
        finally:
            n = lib.axon_stop_nrt_profile(str(output_dir).encode())
            print(f"ntff profile: {n} file(s) -> {output_dir}", file=sys.stderr)

    from antenv.axon_hooks import set_axon_ntff_profile_hook
    set_axon_ntff_profile_hook(_hook)


def kernel(x, qkv_w, q_bias, v_bias, rpb_table, proj_w, proj_b, rel_index):
    global last_exec_time_ns, last_results
    import os
    if os.environ.get("KERNEL_TRACE"):
        _install_ntff_hook()
    from concourse.bass_utils import run_bass_kernel_spmd

    x = np.asarray(x, dtype=np.float32)
    qkv_w = np.asarray(qkv_w, dtype=np.float32)
    q_bias = np.asarray(q_bias, dtype=np.float32)
    v_bias = np.asarray(v_bias, dtype=np.float32)
    rpb_table = np.asarray(rpb_table, dtype=np.float32)
    proj_w = np.asarray(proj_w, dtype=np.float32)
    proj_b = np.asarray(proj_b, dtype=np.float32)

    if 'nc' not in _cache:
        _cache['nc'] = build_program()
    nc = _cache['nc']

    in_maps, bpc = _marshal(x, qkv_w, q_bias, v_bias, rpb_table,
                            proj_w, proj_b, rel_index)
    res = run_bass_kernel_spmd(nc, in_maps, core_ids=list(range(8)),
                               trace=bool(os.environ.get("KERNEL_TRACE")))
    last_exec_time_ns = res.exec_time_ns
    last_results = res
    ys = [res.results[c]["y"].reshape(bpc, N_TOK, DIM) for c in range(8)]
    return np.concatenate(ys, axis=0).astype(np.float32)


# revision 39
# speedup vs baseline: 1.1380x; 1.1380x over previous
"""TRN2 Bass kernel for BEiT-style attention (nn_Attention_27771258536423).

Data-parallel over batch: 8 batches/core. Per core (ntok = 8*197 = 1576):
  P1: qkv projection, all bf16. q,k channel-major [ch, tok] (q pre-scaled
      + biased), v token-major per batch in a head-major [*, 12*65] layout
      with a ones column per head (for softmax denominators).
  P2 per (batch, head-pair): scores computed TRANSPOSED S^T[n2, n1] = K.Q
      -> +bias^T (host-gathered) -> exp (scalar engine, no accum)
      -> AV with E^T as stationary: out[n1, 65] (col 64 = sum = denom)
      -> reciprocal + per-partition scale -> token-major attn [tok, 768]
      -> DMA-transpose engine flips to channel-major attT for P3.
  P3: y = attT.T @ proj_w.T + proj_b, token-major f32 out.
"""
import sys

sys.path.insert(0, '/opt/trn_rl_repo')

import numpy as np
import ml_dtypes

import concourse.bass as bass
import concourse.mybir as mybir
import concourse.tile as tile
from concourse import bacc

dt = mybir.dt
BF16 = ml_dtypes.bfloat16

DIM = 768
NH = 12
HD = 64
N_TOK = 197
SCALE = HD ** (-0.5)
NB = 8                     # batches per core
NTOK = NB * N_TOK          # 1576
QKW = 1664                 # padded qk tile width (13*128)
BSTRIDE = 1664             # attT block stride: 8 batches x 208 (16-aligned slots)
N1C = [(0, 128), (128, 69)]   # token chunks within one batch

_cache = {}


def _ap(t, offset, ap):
    return bass.AP(tensor=t.tensor if hasattr(t, 'tensor') else t,
                   offset=offset, ap=ap)


def build_program():
    nc = bacc.Bacc(None)

    xTb_d = nc.dram_tensor("xTb", [DIM, NTOK], dt.bfloat16, kind="ExternalInput")
    wqkvTb_d = nc.dram_tensor("wqkvTb", [DIM, 3 * DIM], dt.bfloat16, kind="ExternalInput")
    wprojTb_d = nc.dram_tensor("wprojTb", [DIM, DIM], dt.bfloat16, kind="ExternalInput")
    qbs_d = nc.dram_tensor("qbs", [128, 6], dt.float32, kind="ExternalInput")
    vb_d = nc.dram_tensor("vb", [DIM], dt.float32, kind="ExternalInput")
    pb_d = nc.dram_tensor("pb", [DIM], dt.float32, kind="ExternalInput")
    biasTp_d = nc.dram_tensor("biasTp", [6, 128, 1024], dt.bfloat16, kind="ExternalInput")
    y_d = nc.dram_tensor("y", [NTOK, DIM], dt.float32, kind="ExternalOutput")

    Exp = mybir.ActivationFunctionType.Exp
    Ident = mybir.ActivationFunctionType.Identity

    # qkv N-chunks over tokens (394 = 2 batches)
    qkv_nc = [(394 * i, 394) for i in range(4)]

    with tile.TileContext(nc) as tc:
        import contextlib
        with contextlib.ExitStack() as stk:
            consts = stk.enter_context(tc.tile_pool(name="consts", bufs=1))
            qkp = stk.enter_context(tc.tile_pool(name="qkp", bufs=1))
            vp = stk.enter_context(tc.tile_pool(name="vp", bufs=1))
            xwp = stk.enter_context(tc.tile_pool(name="xwp", bufs=1))
            atp = stk.enter_context(tc.tile_pool(name="atp", bufs=1))

            # ---------- constants / inputs ----------
            # small consts first (sync queue), then x on sync / w on scalar
            # queue so the k-ordered accumulation chains pipeline behind the
            # loads
            qbs_sb = consts.tile([128, 6], dt.float32, name="qbs", tag="qbs")
            nc.sync.dma_start(out=qbs_sb[:, :], in_=qbs_d[:, :])
            vb_rep = consts.tile([128, DIM], dt.float32, name="vbrep", tag="vbrep")
            pb_rep = consts.tile([128, DIM], dt.float32, name="pbrep", tag="pbrep")
            x_sb = []
            w_sb = []
            for k in range(6):
                xt = xwp.tile([128, NTOK], dt.bfloat16, name=f"x{k}", tag=f"x{k}")
                x_sb.append(xt)
                wt = xwp.tile([128, 3 * DIM], dt.bfloat16, name=f"w{k}", tag=f"w{k}")
                w_sb.append(wt)
            # chunked loads on three queues so the k-ordered accumulation
            # chains can start as soon as the first chunks land; the scalar
            # queue gets only the k-col loads so the q-copies aren't stuck
            # behind DMA issues
            for k in range(6):
                nc.sync.dma_start(out=x_sb[k][:, 0:788],
                                  in_=xTb_d[128 * k:128 * (k + 1), 0:788])
                nc.gpsimd.dma_start(out=w_sb[k][:, 0:768],
                                    in_=wqkvTb_d[128 * k:128 * (k + 1), 0:768])
                nc.scalar.dma_start(out=w_sb[k][:, 768:1536],
                                    in_=wqkvTb_d[128 * k:128 * (k + 1), 768:1536])
            for k in range(6):
                nc.sync.dma_start(out=x_sb[k][:, 788:NTOK],
                                  in_=xTb_d[128 * k:128 * (k + 1), 788:NTOK])
                nc.gpsimd.dma_start(out=w_sb[k][:, 1536:2304],
                                    in_=wqkvTb_d[128 * k:128 * (k + 1), 1536:2304])
            # broadcast consts late (128-partition replication is slow)
            nc.gpsimd.dma_start(out=vb_rep[:, :],
                                in_=_ap(vb_d, 0, [[0, 128], [1, DIM]]))
            nc.gpsimd.dma_start(out=pb_rep[:, :],
                                in_=_ap(pb_d, 0, [[0, 128], [1, DIM]]))
            biasT_sb = []
            for p in range(6):
                bt = consts.tile([128, 1024], dt.bfloat16, name=f"bT{p}", tag=f"bT{p}")
                nc.sync.dma_start(out=bt[:, :], in_=biasTp_d[p, :, :])
                biasT_sb.append(bt)
            wp_sb = []
            for k in range(6):
                wt = xwp.tile([128, DIM], dt.bfloat16, name=f"wp{k}", tag=f"wp{k}")
                nc.sync.dma_start(out=wt[:, :], in_=wprojTb_d[128 * k:128 * (k + 1), :])
                wp_sb.append(wt)

            # ---------- persistent sbuf ----------
            qk_sb = []   # 0-5 q (ch-major, scaled+biased), 6-11 k (ch-major)
            for t in range(12):
                qk_sb.append(qkp.tile([128, QKW], dt.bfloat16, name=f"qk{t}", tag=f"qk{t}"))
            for t in range(6, 12):   # zero the pad cols read by b=7 extended stat
                nc.gpsimd.memset(qk_sb[t][:, NTOK:QKW], 0.0)
            v65 = []     # per batch: [128, 780] + [69, 780]; head h cols 65h..65h+64, ones at 65h+64
            for b in range(NB):
                v65.append([vp.tile([128, 780], dt.bfloat16, name=f"v{b}_0", tag=f"v{b}_0"),
                            vp.tile([69, 780], dt.bfloat16, name=f"v{b}_1", tag=f"v{b}_1")])
            attT = atp.tile([128, 6 * BSTRIDE], dt.bfloat16, name="attT", tag="attT")

            mps = stk.enter_context(tc.tile_pool(name="mps", bufs=3, space="PSUM"))
            qkps_box = {}

            def emit_qk_tile(m, chunks):
                qkps = qkps_box['p']
                # qk channel tile m: q if m<6 else k; stat = w cols 128m..
                for (no, nw) in chunks:
                    ps = qkps.tile([128, 394], dt.float32, name="qkps", tag="qkps")
                    for k in range(6):
                        nc.tensor.matmul(ps[:, 0:nw],
                                         w_sb[k][:, 128 * m:128 * (m + 1)],
                                         x_sb[k][:, no:no + nw],
                                         start=(k == 0), stop=(k == 5))
                    if m < 6:
                        # q: (ps*SCALE + qb*SCALE) on scalar engine
                        nc.scalar.activation(out=qk_sb[m][:, no:no + nw],
                                             in_=ps[:, 0:nw], func=Ident,
                                             bias=qbs_sb[:, m:m + 1],
                                             scale=float(SCALE))
                    else:
                        nc.vector.tensor_copy(qk_sb[m][:, no:no + nw], ps[:, 0:nw])

            def emit_v65(b, part):
                # part 0: ci=0 halves; part 1: ci=1 halves (+ ones memsets)
                ci = part
                to, tw_ = N1C[ci]
                vt = v65[b][ci]
                for half in range(2):
                    ps = mps.tile([128, 384], dt.float32, name="mp", tag="mp")
                    for k in range(6):
                        nc.tensor.matmul(
                            ps[0:tw_, 0:384],
                            x_sb[k][:, N_TOK * b + to:N_TOK * b + to + tw_],
                            w_sb[k][:, 1536 + 384 * half:1536 + 384 * (half + 1)],
                            start=(k == 0), stop=(k == 5))
                    # add v bias, scatter to head-major 65-stride cols
                    out_ap = _ap(vt, vt.offset + 65 * 6 * half,
                                 [[vt.ap[0][0], tw_], [65, 6], [1, 64]])
                    in1_ap = _ap(vb_rep, vb_rep.offset + 384 * half,
                                 [[vb_rep.ap[0][0], tw_], [64, 6], [1, 64]])
                    nc.vector.tensor_tensor(out=out_ap, in0=ps[0:tw_, 0:384],
                                            in1=in1_ap, op=mybir.AluOpType.add)
                # ones columns for this tile
                ones_ap = _ap(vt, vt.offset + 64, [[vt.ap[0][0], tw_], [65, 12]])
                nc.gpsimd.memset(ones_ap, 1.0)

            # ---------- P1: qk (+ first v65 batches) ----------
            with tc.tile_pool(name="qkps", bufs=4, space="PSUM") as qkps:
                qkps_box['p'] = qkps
                # token chunks 0-1 for all tiles first (wave-1 x data), then
                # chunks 2-3 once the second-wave DMAs have landed
                for j in range(6):
                    emit_qk_tile(j, qkv_nc[0:2])
                    emit_qk_tile(j + 6, qkv_nc[0:2])
                for j in range(6):
                    emit_qk_tile(j, qkv_nc[2:4])
                    emit_qk_tile(j + 6, qkv_nc[2:4])
                for b in range(2):
                    emit_v65(b, 0)
                    emit_v65(b, 1)

            # ---------- P2 + interleaved v65/P3 ----------
            spool = stk.enter_context(tc.tile_pool(name="spool", bufs=5, space="PSUM"))
            ssp = stk.enter_context(tc.tile_pool(name="ssp", bufs=6))
            etp = stk.enter_context(tc.tile_pool(name="etp", bufs=6))
            recp = stk.enter_context(tc.tile_pool(name="recp", bufs=6))
            attp = stk.enter_context(tc.tile_pool(name="attp", bufs=2))
            ysb = stk.enter_context(tc.tile_pool(name="ysb", bufs=3))

            def unit_head(b, h, att2):
                p, i = h // 2, h % 2
                qt = qk_sb[p]
                kt = qk_sb[6 + p]
                po = 64 * i
                sp = spool.tile([128, 512], dt.float32, name="sp", tag="sp")
                for ci in range(2):
                    nc.tensor.matmul(
                        sp[0:128, 256 * ci:256 * ci + N_TOK],
                        kt[po:po + 64,
                           N_TOK * b + 128 * ci:N_TOK * b + 128 * (ci + 1)],
                        qt[po:po + 64, N_TOK * b:N_TOK * b + N_TOK],
                        start=True, stop=True)
                # exp of raw scores (scalar engine, psum -> bf16 sbuf)
                es = ssp.tile([128, 512], dt.bfloat16, name="es", tag="es")
                sp_ap = _ap(sp, sp.offset, [[sp.ap[0][0], 128], [256, 2], [1, N_TOK]])
                es_ap = _ap(es, es.offset, [[es.ap[0][0], 128], [256, 2], [1, N_TOK]])
                nc.scalar.activation(out=es_ap, in_=sp_ap, func=Exp)
                # fold bias in via exp(s+b) = exp(s)*exp(b); all-bf16 sbuf op,
                # EBT precomputed on host
                bt = biasT_sb[p]
                et = etp.tile([128, 512], dt.bfloat16, name="et", tag="et")
                et_ap = _ap(et, et.offset, [[et.ap[0][0], 128], [256, 2], [1, N_TOK]])
                es_ap2 = _ap(es, es.offset, [[es.ap[0][0], 128], [256, 2], [1, N_TOK]])
                bt_ap = _ap(bt, bt.offset + 512 * i,
                            [[bt.ap[0][0], 128], [256, 2], [1, N_TOK]])
                eng = nc.vector
                eng.tensor_tensor(out=et_ap, in0=es_ap2, in1=bt_ap,
                                  op=mybir.AluOpType.mult)
                # AV: out[n1, 65] (col 64 = denom); psum shared with v65/P3
                ap_ = mps.tile([128, 384], dt.float32, name="mp", tag="mp")
                for cj, (n1o, n1c) in enumerate(N1C):
                    for ci, (n2o, n2c) in enumerate(N1C):
                        nc.tensor.matmul(
                            ap_[0:n1c, 65 * cj:65 * cj + 65],
                            et[0:n2c, 256 * ci + 128 * cj:256 * ci + 128 * cj + n1c],
                            v65[b][ci][0:n2c, 65 * h:65 * h + 65],
                            start=(ci == 0), stop=(ci == 1))
                rec = recp.tile([128, 2], dt.float32, name="rec", tag="rec")
                nc.vector.reciprocal(
                    rec[:, 0:2],
                    _ap(ap_, ap_.offset + 64, [[ap_.ap[0][0], 128], [65, 2]]))
                for cj, (n1o, n1c) in enumerate(N1C):
                    src = ap_[0:n1c, 65 * cj:65 * cj + 64]
                    dst = att2[cj][0:n1c, 64 * h:64 * h + 64]
                    if (b + h + cj) % 2 == 0:
                        nc.scalar.mul(dst, src, rec[0:n1c, cj:cj + 1])
                    else:
                        nc.vector.tensor_scalar(
                            out=dst, in0=src, scalar1=rec[0:n1c, cj:cj + 1],
                            scalar2=None, op0=mybir.AluOpType.mult)

            def emit_p3(b, ci):
                to, tw_ = N1C[ci]
                ys = ysb.tile([128, DIM], dt.float32, name="ys", tag="ys")
                for half in range(2):
                    pp = mps.tile([128, 384], dt.float32, name="mp", tag="mp")
                    for c in range(6):
                        nc.tensor.matmul(
                            pp[0:tw_, 0:384],
                            attT[0:128,
                                 BSTRIDE * c + 208 * b + to:
                                 BSTRIDE * c + 208 * b + to + tw_],
                            wp_sb[c][0:128, 384 * half:384 * (half + 1)],
                            start=(c == 0), stop=(c == 5))
                    nc.vector.tensor_tensor(
                        out=ys[0:tw_, 384 * half:384 * (half + 1)],
                        in0=pp[0:tw_, 0:384],
                        in1=pb_rep[0:tw_, 384 * half:384 * (half + 1)],
                        op=mybir.AluOpType.add)
                nc.sync.dma_start(out=y_d[N_TOK * b + to:N_TOK * b + to + tw_, :],
                                  in_=ys[0:tw_, :])

            for b in range(NB):
                att2 = [attp.tile([128, DIM], dt.bfloat16, name="att0", tag="att0"),
                        attp.tile([80, DIM], dt.bfloat16, name="att1", tag="att1")]
                # pad rows 69:80 of att1 (transpose reads them); partition
                # slices must start at 0/32/64/96, so clear the whole tile
                # before the scale-copies fill rows 0:69
                nc.gpsimd.memset(att2[1][0:80, :], 0.0)
                for h in range(12):
                    unit_head(b, h, att2)
                    # P3 of the previous batch goes here so the in-order PE
                    # queue doesn't stall on the previous transpose DMA
                    if h == 2 and b > 0:
                        emit_p3(b - 1, 0)
                    elif h == 5 and b > 0:
                        emit_p3(b - 1, 1)
                    if b + 2 < NB:
                        if h == 3:
                            emit_v65(b + 2, 0)
                        elif h == 7:
                            emit_v65(b + 2, 1)
                    if b == NB - 1 and h % 2 == 1:
                        # last batch: transpose block c as soon as its two
                        # heads are done, shrinking the end-of-kernel drain
                        c = h // 2
                        oc0 = _ap(attT, attT.offset + BSTRIDE * c + 208 * b,
                                  [[attT.ap[0][0], 128], [1, 128]])
                        nc.sync.dma_start_transpose(
                            out=oc0, in_=att2[0][0:128, 128 * c:128 * (c + 1)])
                        oc1 = _ap(attT, attT.offset + BSTRIDE * c + 208 * b + 128,
                                  [[attT.ap[0][0], 128], [1, 80]])
                        nc.sync.dma_start_transpose(
                            out=oc1, in_=att2[1][0:80, 128 * c:128 * (c + 1)])
                if b < NB - 1:
                    # transpose token-major att -> channel-major attT blocks
                    # (dst token offsets must be 16-aligned: 208*b, 208*b+128)
                    out0 = _ap(attT, attT.offset + 208 * b,
                               [[attT.ap[0][0], 128], [BSTRIDE, 6], [1, 128]])
                    nc.sync.dma_start_transpose(out=out0, in_=att2[0][0:128, :])
                    out1 = _ap(attT, attT.offset + 208 * b + 128,
                               [[attT.ap[0][0], 128], [BSTRIDE, 6], [1, 80]])
                    nc.sync.dma_start_transpose(out=out1, in_=att2[1][0:80, :])
            emit_p3(NB - 1, 0)
            emit_p3(NB - 1, 1)

    nc.compile()
    return nc


def _marshal(x, qkv_w, q_bias, v_bias, rpb_table, proj_w, proj_b, rel_index):
    B = x.shape[0]
    ncore = 8
    bpc = B // ncore
    x2 = np.ascontiguousarray(x.reshape(B, N_TOK, DIM))

    wqkvTb = np.ascontiguousarray(qkv_w.T.astype(BF16))
    wprojTb = np.ascontiguousarray(proj_w.T.astype(BF16))
    qbs = np.ascontiguousarray((q_bias * SCALE).astype(np.float32).reshape(6, 128).T)

    # host-side gather of exp(relative-position bias), packed transposed:
    # per head pair p: [128, 1024] with chunks at col 0/256 (h=2p) and 512/768.
    # Shipping exp(b) lets the device fold the bias as exp(s)*exp(b) with a
    # cheap all-bf16 multiply instead of a psum f32 add.
    Bfull = np.exp(rpb_table.astype(np.float64))[np.asarray(rel_index)]  # [197,197,12]
    biasTp = np.zeros((6, 128, 1024), dtype=BF16)
    for p in range(6):
        for i in range(2):
            bT = Bfull[:, :, 2 * p + i].T               # [n2, n1]
            biasTp[p, 0:128, 512 * i:512 * i + N_TOK] = bT[0:128, :].astype(BF16)
            biasTp[p, 0:69, 512 * i + 256:512 * i + 256 + N_TOK] = bT[128:197, :].astype(BF16)

    shared = {"wqkvTb": wqkvTb, "wprojTb": wprojTb, "qbs": qbs,
              "vb": np.ascontiguousarray(v_bias.astype(np.float32)),
              "pb": np.ascontiguousarray(proj_b.astype(np.float32)),
              "biasTp": biasTp}
    in_maps = []
    for c in range(ncore):
        xTb = np.ascontiguousarray(
            x2[c * bpc:(c + 1) * bpc].reshape(bpc * N_TOK, DIM).T.astype(BF16))
        m = dict(shared)
        m["xTb"] = xTb
        in_maps.append(m)
    return in_maps, bpc


last_exec_time_ns = None
last_results = None


def _install_ntff_hook():
    """Provide antenv.axon_hooks + register the ctypes NTFF hook (the agent
    image's antenv lacks axon_hooks, so trn_boot degraded silently)."""
    import types
    import contextlib
    import ctypes

    try:
        from antenv.axon_hooks import get_axon_ntff_profile_hook
        if get_axon_ntff_profile_hook() is not None:
            return
    except ImportError:
        import antenv
        mod = types.ModuleType("antenv.axon_hooks")
        mod._hook = None

        def set_axon_ntff_profile_hook(h):
            mod._hook = h

        def get_axon_ntff_profile_hook():
            return mod._hook

        mod.set_axon_ntff_profile_hook = set_axon_ntff_profile_hook
        mod.get_axon_ntff_profile_hook = get_axon_ntff_profile_hook
        sys.modules["antenv.axon_hooks"] = mod
        antenv.axon_hooks = mod

    so_path = "/opt/axon/libaxon_pjrt.so"
    lib = ctypes.CDLL(so_path)
    if not hasattr(lib, "axon_start_nrt_profile"):
        return
    lib.axon_start_nrt_profile.argtypes = [ctypes.POINTER(ctypes.c_int64),
                                           ctypes.c_size_t]
    lib.axon_start_nrt_profile.restype = ctypes.c_int64
    lib.axon_stop_nrt_profile.argtypes = [ctypes.c_char_p]
    lib.axon_stop_nrt_profile.restype = ctypes.c_int64

    @contextlib.contextmanager
    def _hook(output_dir, device_ids):
        import jax
        jax.devices()
        if device_ids:
            ids = (ctypes.c_int64 * len(device_ids))(*device_ids)
            rc = lib.axon_start_nrt_profile(ids, len(device_ids))
        else:
            rc = lib.axon_start_nrt_profile(None, 0)
        if rc != 0:
            raise RuntimeError(f"axon_start_nrt_profile rc={rc}")
        try:
            yield
        finally:
            n = lib.axon_stop_nrt_profile(str(output_dir).encode())
            print(f"ntff profile: {n} file(s) -> {output_dir}", file=sys.stderr)

    from antenv.axon_hooks import set_axon_ntff_profile_hook
    set_axon_ntff_profile_hook(_hook)


def kernel(x, qkv_w, q_bias, v_bias, rpb_table, proj_w, proj_b, rel_index):
    global last_exec_time_ns, last_results
    import os
    if os.environ.get("KERNEL_TRACE"):
        _install_ntff_hook()
    from concourse.bass_utils import run_bass_kernel_spmd

    x = np.asarray(x, dtype=np.float32)
    qkv_w = np.asarray(qkv_w, dtype=np.float32)
    q_bias = np.asarray(q_bias, dtype=np.float32)
    v_bias = np.asarray(v_bias, dtype=np.float32)
    rpb_table = np.asarray(rpb_table, dtype=np.float32)
    proj_w = np.asarray(proj_w, dtype=np.float32)
    proj_b = np.asarray(proj_b, dtype=np.float32)

    if 'nc' not in _cache:
        _cache['nc'] = build_program()
    nc = _cache['nc']

    in_maps, bpc = _marshal(x, qkv_w, q_bias, v_bias, rpb_table,
                            proj_w, proj_b, rel_index)
    res = run_bass_kernel_spmd(nc, in_maps, core_ids=list(range(8)),
                               trace=bool(os.environ.get("KERNEL_TRACE")))
    last_exec_time_ns = res.exec_time_ns
    last_results = res
    ys = [res.results[c]["y"].reshape(bpc, N_TOK, DIM) for c in range(8)]
    return np.concatenate(ys, axis=0).astype(np.float32)


# revision 41
# speedup vs baseline: 1.1698x; 1.0280x over previous
"""TRN2 Bass kernel for BEiT-style attention (nn_Attention_27771258536423).

Data-parallel over batch: 8 batches/core. Per core (ntok = 8*197 = 1576):
  P1: qkv projection, all bf16. q,k channel-major [ch, tok] (q pre-scaled
      + biased), v token-major per batch in a head-major [*, 12*65] layout
      with a ones column per head (for softmax denominators).
  P2 per (batch, head-pair): scores computed TRANSPOSED S^T[n2, n1] = K.Q
      -> +bias^T (host-gathered) -> exp (scalar engine, no accum)
      -> AV with E^T as stationary: out[n1, 65] (col 64 = sum = denom)
      -> reciprocal + per-partition scale -> token-major attn [tok, 768]
      -> DMA-transpose engine flips to channel-major attT for P3.
  P3: y = attT.T @ proj_w.T + proj_b, token-major f32 out.
"""
import sys

sys.path.insert(0, '/opt/trn_rl_repo')

import numpy as np
import ml_dtypes

import concourse.bass as bass
import concourse.mybir as mybir
import concourse.tile as tile
from concourse import bacc

dt = mybir.dt
BF16 = ml_dtypes.bfloat16

DIM = 768
NH = 12
HD = 64
N_TOK = 197
SCALE = HD ** (-0.5)
NB = 8                     # batches per core
NTOK = NB * N_TOK          # 1576
QKW = 1664                 # padded qk tile width (13*128)
BSTRIDE = 1664             # attT block stride: 8 batches x 208 (16-aligned slots)
N1C = [(0, 128), (128, 69)]   # token chunks within one batch

_cache = {}


def _ap(t, offset, ap):
    return bass.AP(tensor=t.tensor if hasattr(t, 'tensor') else t,
                   offset=offset, ap=ap)


def build_program():
    nc = bacc.Bacc(None)

    xTb_d = nc.dram_tensor("xTb", [DIM, NTOK], dt.bfloat16, kind="ExternalInput")
    wqkvTb_d = nc.dram_tensor("wqkvTb", [DIM, 3 * DIM], dt.bfloat16, kind="ExternalInput")
    wprojTb_d = nc.dram_tensor("wprojTb", [DIM, DIM], dt.bfloat16, kind="ExternalInput")
    qbs_d = nc.dram_tensor("qbs", [128, 6], dt.float32, kind="ExternalInput")
    vb_d = nc.dram_tensor("vb", [DIM], dt.float32, kind="ExternalInput")
    pb_d = nc.dram_tensor("pb", [DIM], dt.float32, kind="ExternalInput")
    biasTp_d = nc.dram_tensor("biasTp", [6, 128, 1024], dt.bfloat16, kind="ExternalInput")
    y_d = nc.dram_tensor("y", [NTOK, DIM], dt.float32, kind="ExternalOutput")

    Exp = mybir.ActivationFunctionType.Exp
    Ident = mybir.ActivationFunctionType.Identity

    # qkv N-chunks over tokens (394 = 2 batches)
    qkv_nc = [(394 * i, 394) for i in range(4)]

    with tile.TileContext(nc) as tc:
        import contextlib
        with contextlib.ExitStack() as stk:
            consts = stk.enter_context(tc.tile_pool(name="consts", bufs=1))
            qkp = stk.enter_context(tc.tile_pool(name="qkp", bufs=1))
            vp = stk.enter_context(tc.tile_pool(name="vp", bufs=1))
            xwp = stk.enter_context(tc.tile_pool(name="xwp", bufs=1))
            atp = stk.enter_context(tc.tile_pool(name="atp", bufs=1))

            # ---------- constants / inputs ----------
            # small consts first (sync queue), then x on sync / w on scalar
            # queue so the k-ordered accumulation chains pipeline behind the
            # loads
            qbs_sb = consts.tile([128, 6], dt.float32, name="qbs", tag="qbs")
            nc.sync.dma_start(out=qbs_sb[:, :], in_=qbs_d[:, :])
            vb_rep = consts.tile([128, DIM], dt.float32, name="vbrep", tag="vbrep")
            pb_rep = consts.tile([128, DIM], dt.float32, name="pbrep", tag="pbrep")
            x_sb = []
            w_sb = []
            for k in range(6):
                xt = xwp.tile([128, NTOK], dt.bfloat16, name=f"x{k}", tag=f"x{k}")
                x_sb.append(xt)
                wt = xwp.tile([128, 3 * DIM], dt.bfloat16, name=f"w{k}", tag=f"w{k}")
                w_sb.append(wt)
            # chunked loads on three queues so the k-ordered accumulation
            # chains can start as soon as the first chunks land; the scalar
            # queue gets only the k-col loads so the q-copies aren't stuck
            # behind DMA issues
            for k in range(6):
                nc.sync.dma_start(out=x_sb[k][:, 0:788],
                                  in_=xTb_d[128 * k:128 * (k + 1), 0:788])
                nc.gpsimd.dma_start(out=w_sb[k][:, 0:768],
                                    in_=wqkvTb_d[128 * k:128 * (k + 1), 0:768])
                nc.scalar.dma_start(out=w_sb[k][:, 768:1536],
                                    in_=wqkvTb_d[128 * k:128 * (k + 1), 768:1536])
            for k in range(6):
                nc.sync.dma_start(out=x_sb[k][:, 788:NTOK],
                                  in_=xTb_d[128 * k:128 * (k + 1), 788:NTOK])
                nc.gpsimd.dma_start(out=w_sb[k][:, 1536:2304],
                                    in_=wqkvTb_d[128 * k:128 * (k + 1), 1536:2304])
            # broadcast consts late (128-partition replication is slow)
            nc.gpsimd.dma_start(out=vb_rep[:, :],
                                in_=_ap(vb_d, 0, [[0, 128], [1, DIM]]))
            nc.gpsimd.dma_start(out=pb_rep[:, :],
                                in_=_ap(pb_d, 0, [[0, 128], [1, DIM]]))
            wp_sb = []
            for k in range(6):
                wt = xwp.tile([128, DIM], dt.bfloat16, name=f"wp{k}", tag=f"wp{k}")
                nc.scalar.dma_start(out=wt[:, :], in_=wprojTb_d[128 * k:128 * (k + 1), :])
                wp_sb.append(wt)
            biasT_sb = []
            for p in range(6):
                bt = consts.tile([128, 1024], dt.bfloat16, name=f"bT{p}", tag=f"bT{p}")
                nc.scalar.dma_start(out=bt[:, :], in_=biasTp_d[p, :, :])
                biasT_sb.append(bt)

            # ---------- persistent sbuf ----------
            qk_sb = []   # 0-5 q (ch-major, scaled+biased), 6-11 k (ch-major)
            for t in range(12):
                qk_sb.append(qkp.tile([128, QKW], dt.bfloat16, name=f"qk{t}", tag=f"qk{t}"))
            for t in range(6, 12):   # zero the pad cols read by b=7 extended stat
                nc.gpsimd.memset(qk_sb[t][:, NTOK:QKW], 0.0)
            v65 = []     # per batch: [128, 780] + [69, 780]; head h cols 65h..65h+64, ones at 65h+64
            for b in range(NB):
                v65.append([vp.tile([128, 780], dt.bfloat16, name=f"v{b}_0", tag=f"v{b}_0"),
                            vp.tile([69, 780], dt.bfloat16, name=f"v{b}_1", tag=f"v{b}_1")])
            attT = atp.tile([128, 6 * BSTRIDE], dt.bfloat16, name="attT", tag="attT")

            mps = stk.enter_context(tc.tile_pool(name="mps", bufs=3, space="PSUM"))
            qkps_box = {}

            def emit_qk_tile(m, chunks):
                qkps = qkps_box['p']
                # qk channel tile m: q if m<6 else k; stat = w cols 128m..
                for (no, nw) in chunks:
                    ps = qkps.tile([128, 394], dt.float32, name="qkps", tag="qkps")
                    for k in range(6):
                        nc.tensor.matmul(ps[:, 0:nw],
                                         w_sb[k][:, 128 * m:128 * (m + 1)],
                                         x_sb[k][:, no:no + nw],
                                         start=(k == 0), stop=(k == 5))
                    if m < 6:
                        # q: (ps*SCALE + qb*SCALE) on scalar engine
                        nc.scalar.activation(out=qk_sb[m][:, no:no + nw],
                                             in_=ps[:, 0:nw], func=Ident,
                                             bias=qbs_sb[:, m:m + 1],
                                             scale=float(SCALE))
                    else:
                        nc.vector.tensor_copy(qk_sb[m][:, no:no + nw], ps[:, 0:nw])

            def emit_v65(b, part):
                # part 0: ci=0 halves; part 1: ci=1 halves (+ ones memsets)
                ci = part
                to, tw_ = N1C[ci]
                vt = v65[b][ci]
                for half in range(2):
                    ps = mps.tile([128, 384], dt.float32, name="mp", tag="mp")
                    for k in range(6):
                        nc.tensor.matmul(
                            ps[0:tw_, 0:384],
                            x_sb[k][:, N_TOK * b + to:N_TOK * b + to + tw_],
                            w_sb[k][:, 1536 + 384 * half:1536 + 384 * (half + 1)],
                            start=(k == 0), stop=(k == 5))
                    # add v bias, scatter to head-major 65-stride cols
                    out_ap = _ap(vt, vt.offset + 65 * 6 * half,
                                 [[vt.ap[0][0], tw_], [65, 6], [1, 64]])
                    in1_ap = _ap(vb_rep, vb_rep.offset + 384 * half,
                                 [[vb_rep.ap[0][0], tw_], [64, 6], [1, 64]])
                    nc.vector.tensor_tensor(out=out_ap, in0=ps[0:tw_, 0:384],
                                            in1=in1_ap, op=mybir.AluOpType.add)
                # ones columns for this tile
                ones_ap = _ap(vt, vt.offset + 64, [[vt.ap[0][0], tw_], [65, 12]])
                nc.gpsimd.memset(ones_ap, 1.0)

            # ---------- P1: qk (+ first v65 batches) ----------
            with tc.tile_pool(name="qkps", bufs=4, space="PSUM") as qkps:
                qkps_box['p'] = qkps
                # token chunks 0-1 for all tiles first (wave-1 x data), then
                # chunks 2-3 once the second-wave DMAs have landed
                for j in range(6):
                    emit_qk_tile(j, qkv_nc[0:2])
                    emit_qk_tile(j + 6, qkv_nc[0:2])
                for j in range(6):
                    emit_qk_tile(j, qkv_nc[2:4])
                    emit_qk_tile(j + 6, qkv_nc[2:4])
                for b in range(2):
                    emit_v65(b, 0)
                    emit_v65(b, 1)

            # ---------- P2 + interleaved v65/P3 ----------
            spool = stk.enter_context(tc.tile_pool(name="spool", bufs=5, space="PSUM"))
            ssp = stk.enter_context(tc.tile_pool(name="ssp", bufs=6))
            etp = stk.enter_context(tc.tile_pool(name="etp", bufs=6))
            recp = stk.enter_context(tc.tile_pool(name="recp", bufs=6))
            attp = stk.enter_context(tc.tile_pool(name="attp", bufs=2))
            ysb = stk.enter_context(tc.tile_pool(name="ysb", bufs=3))

            def unit_head(b, h, att2):
                p, i = h // 2, h % 2
                qt = qk_sb[p]
                kt = qk_sb[6 + p]
                po = 64 * i
                sp = spool.tile([128, 512], dt.float32, name="sp", tag="sp")
                for ci in range(2):
                    nc.tensor.matmul(
                        sp[0:128, 256 * ci:256 * ci + N_TOK],
                        kt[po:po + 64,
                           N_TOK * b + 128 * ci:N_TOK * b + 128 * (ci + 1)],
                        qt[po:po + 64, N_TOK * b:N_TOK * b + N_TOK],
                        start=True, stop=True)
                # exp of raw scores (scalar engine, psum -> bf16 sbuf)
                es = ssp.tile([128, 512], dt.bfloat16, name="es", tag="es")
                sp_ap = _ap(sp, sp.offset, [[sp.ap[0][0], 128], [256, 2], [1, N_TOK]])
                es_ap = _ap(es, es.offset, [[es.ap[0][0], 128], [256, 2], [1, N_TOK]])
                nc.scalar.activation(out=es_ap, in_=sp_ap, func=Exp)
                # fold bias in via exp(s+b) = exp(s)*exp(b); all-bf16 sbuf op,
                # EBT precomputed on host
                bt = biasT_sb[p]
                et = etp.tile([128, 512], dt.bfloat16, name="et", tag="et")
                et_ap = _ap(et, et.offset, [[et.ap[0][0], 128], [256, 2], [1, N_TOK]])
                es_ap2 = _ap(es, es.offset, [[es.ap[0][0], 128], [256, 2], [1, N_TOK]])
                bt_ap = _ap(bt, bt.offset + 512 * i,
                            [[bt.ap[0][0], 128], [256, 2], [1, N_TOK]])
                eng = nc.vector
                eng.tensor_tensor(out=et_ap, in0=es_ap2, in1=bt_ap,
                                  op=mybir.AluOpType.mult)
                # AV: out[n1, 65] (col 64 = denom); psum shared with v65/P3
                ap_ = mps.tile([128, 384], dt.float32, name="mp", tag="mp")
                for cj, (n1o, n1c) in enumerate(N1C):
                    for ci, (n2o, n2c) in enumerate(N1C):
                        nc.tensor.matmul(
                            ap_[0:n1c, 65 * cj:65 * cj + 65],
                            et[0:n2c, 256 * ci + 128 * cj:256 * ci + 128 * cj + n1c],
                            v65[b][ci][0:n2c, 65 * h:65 * h + 65],
                            start=(ci == 0), stop=(ci == 1))
                rec = recp.tile([128, 2], dt.float32, name="rec", tag="rec")
                nc.vector.reciprocal(
                    rec[:, 0:2],
                    _ap(ap_, ap_.offset + 64, [[ap_.ap[0][0], 128], [65, 2]]))
                for cj, (n1o, n1c) in enumerate(N1C):
                    src = ap_[0:n1c, 65 * cj:65 * cj + 64]
                    dst = att2[cj][0:n1c, 64 * h:64 * h + 64]
                    if (b + h + cj) % 2 == 0:
                        nc.scalar.mul(dst, src, rec[0:n1c, cj:cj + 1])
                    else:
                        nc.vector.tensor_scalar(
                            out=dst, in0=src, scalar1=rec[0:n1c, cj:cj + 1],
                            scalar2=None, op0=mybir.AluOpType.mult)

            def emit_p3(b, ci):
                to, tw_ = N1C[ci]
                ys = ysb.tile([128, DIM], dt.float32, name="ys", tag="ys")
                for half in range(2):
                    pp = mps.tile([128, 384], dt.float32, name="mp", tag="mp")
                    for c in range(6):
                        nc.tensor.matmul(
                            pp[0:tw_, 0:384],
                            attT[0:128,
                                 BSTRIDE * c + 208 * b + to:
                                 BSTRIDE * c + 208 * b + to + tw_],
                            wp_sb[c][0:128, 384 * half:384 * (half + 1)],
                            start=(c == 0), stop=(c == 5))
                    nc.vector.tensor_tensor(
                        out=ys[0:tw_, 384 * half:384 * (half + 1)],
                        in0=pp[0:tw_, 0:384],
                        in1=pb_rep[0:tw_, 384 * half:384 * (half + 1)],
                        op=mybir.AluOpType.add)
                nc.sync.dma_start(out=y_d[N_TOK * b + to:N_TOK * b + to + tw_, :],
                                  in_=ys[0:tw_, :])

            for b in range(NB):
                att2 = [attp.tile([128, DIM], dt.bfloat16, name="att0", tag="att0"),
                        attp.tile([80, DIM], dt.bfloat16, name="att1", tag="att1")]
                # pad rows 69:80 of att1 (transpose reads them); partition
                # slices must start at 0/32/64/96, so clear the whole tile
                # before the scale-copies fill rows 0:69
                nc.gpsimd.memset(att2[1][0:80, :], 0.0)
                for h in range(12):
                    unit_head(b, h, att2)
                    # P3 of the previous batch goes here so the in-order PE
                    # queue doesn't stall on the previous transpose DMA
                    if h == 2 and b > 0:
                        emit_p3(b - 1, 0)
                    elif h == 5 and b > 0:
                        emit_p3(b - 1, 1)
                    if b + 2 < NB:
                        if h == 3:
                            emit_v65(b + 2, 0)
                        elif h == 7:
                            emit_v65(b + 2, 1)
                # transpose token-major att -> channel-major attT blocks
                # (dst token offsets must be 16-aligned: 208*b and 208*b+128)
                out0 = _ap(attT, attT.offset + 208 * b,
                           [[attT.ap[0][0], 128], [BSTRIDE, 6], [1, 128]])
                nc.sync.dma_start_transpose(out=out0, in_=att2[0][0:128, :])
                out1 = _ap(attT, attT.offset + 208 * b + 128,
                           [[attT.ap[0][0], 128], [BSTRIDE, 6], [1, 80]])
                nc.sync.dma_start_transpose(out=out1, in_=att2[1][0:80, :])
            emit_p3(NB - 1, 0)
            emit_p3(NB - 1, 1)

    nc.compile()
    return nc


def _marshal(x, qkv_w, q_bias, v_bias, rpb_table, proj_w, proj_b, rel_index):
    B = x.shape[0]
    ncore = 8
    bpc = B // ncore
    x2 = np.ascontiguousarray(x.reshape(B, N_TOK, DIM))

    wqkvTb = np.ascontiguousarray(qkv_w.T.astype(BF16))
    wprojTb = np.ascontiguousarray(proj_w.T.astype(BF16))
    qbs = np.ascontiguousarray((q_bias * SCALE).astype(np.float32).reshape(6, 128).T)

    # host-side gather of exp(relative-position bias), packed transposed:
    # per head pair p: [128, 1024] with chunks at col 0/256 (h=2p) and 512/768.
    # Shipping exp(b) lets the device fold the bias as exp(s)*exp(b) with a
    # cheap all-bf16 multiply instead of a psum f32 add.
    Bfull = np.exp(rpb_table.astype(np.float64))[np.asarray(rel_index)]  # [197,197,12]
    biasTp = np.zeros((6, 128, 1024), dtype=BF16)
    for p in range(6):
        for i in range(2):
            bT = Bfull[:, :, 2 * p + i].T               # [n2, n1]
            biasTp[p, 0:128, 512 * i:512 * i + N_TOK] = bT[0:128, :].astype(BF16)
            biasTp[p, 0:69, 512 * i + 256:512 * i + 256 + N_TOK] = bT[128:197, :].astype(BF16)

    shared = {"wqkvTb": wqkvTb, "wprojTb": wprojTb, "qbs": qbs,
              "vb": np.ascontiguousarray(v_bias.astype(np.float32)),
              "pb": np.ascontiguousarray(proj_b.astype(np.float32)),
              "biasTp": biasTp}
    in_maps = []
    for c in range(ncore):
        xTb = np.ascontiguousarray(
            x2[c * bpc:(c + 1) * bpc].reshape(bpc * N_TOK, DIM).T.astype(BF16))
        m = dict(shared)
        m["xTb"] = xTb
        in_maps.append(m)
    return in_maps, bpc


last_exec_time_ns = None
last_results = None


def _install_ntff_hook():
    """Provide antenv.axon_hooks + register the ctypes NTFF hook (the agent
    image's antenv lacks axon_hooks, so trn_boot degraded silently)."""
    import types
    import contextlib
    import ctypes

    try:
        from antenv.axon_hooks import get_axon_ntff_profile_hook
        if get_axon_ntff_profile_hook() is not None:
            return
    except ImportError:
        import antenv
        mod = types.ModuleType("antenv.axon_hooks")
        mod._hook = None

        def set_axon_ntff_profile_hook(h):
            mod._hook = h

        def get_axon_ntff_profile_hook():
            return mod._hook

        mod.set_axon_ntff_profile_hook = set_axon_ntff_profile_hook
        mod.get_axon_ntff_profile_hook = get_axon_ntff_profile_hook
        sys.modules["antenv.axon_hooks"] = mod
        antenv.axon_hooks = mod

    so_path = "/opt/axon/libaxon_pjrt.so"
    lib = ctypes.CDLL(so_path)
    if not hasattr(lib, "axon_start_nrt_profile"):
        return
    lib.axon_start_nrt_profile.argtypes = [ctypes.POINTER(ctypes.c_int64),
                                           ctypes.c_size_t]
    lib.axon_start_nrt_profile.restype = ctypes.c_int64
    lib.axon_stop_nrt_profile.argtypes = [ctypes.c_char_p]
    lib.axon_stop_nrt_profile.restype = ctypes.c_int64

    @contextlib.contextmanager
    def _hook(output_dir, device_ids):
        import jax
        jax.devices()
        if device_ids:
            ids = (ctypes.c_int64 * len(device_ids))(*device_ids)
            rc = lib.axon_start_nrt_profile(ids, len(device_ids))
        else:
            rc = lib.axon_start_nrt_profile(None, 0)
        if rc != 0:
            raise RuntimeError(f"axon_start_nrt_profile rc={rc}")
        try:
            yield
        finally:
            n = lib.axon_stop_nrt_profile(str(output_dir).encode())
            print(f"ntff profile: {n} file(s) -> {output_dir}", file=sys.stderr)

    from antenv.axon_hooks import set_axon_ntff_profile_hook
    set_axon_ntff_profile_hook(_hook)


def kernel(x, qkv_w, q_bias, v_bias, rpb_table, proj_w, proj_b, rel_index):
    global last_exec_time_ns, last_results
    import os
    if os.environ.get("KERNEL_TRACE"):
        _install_ntff_hook()
    from concourse.bass_utils import run_bass_kernel_spmd

    x = np.asarray(x, dtype=np.float32)
    qkv_w = np.asarray(qkv_w, dtype=np.float32)
    q_bias = np.asarray(q_bias, dtype=np.float32)
    v_bias = np.asarray(v_bias, dtype=np.float32)
    rpb_table = np.asarray(rpb_table, dtype=np.float32)
    proj_w = np.asarray(proj_w, dtype=np.float32)
    proj_b = np.asarray(proj_b, dtype=np.float32)

    if 'nc' not in _cache:
        _cache['nc'] = build_program()
    nc = _cache['nc']

    in_maps, bpc = _marshal(x, qkv_w, q_bias, v_bias, rpb_table,
                            proj_w, proj_b, rel_index)
    res = run_bass_kernel_spmd(nc, in_maps, core_ids=list(range(8)),
                               trace=bool(os.environ.get("KERNEL_TRACE")))
    last_exec_time_ns = res.exec_time_ns
    last_results = res
    ys = [res.results[c]["y"].reshape(bpc, N_TOK, DIM) for c in range(8)]
    return np.concatenate(ys, axis=0).astype(np.float32)
